# revision 6
# baseline (speedup 1.0000x reference)
"""HGNN conv distributed Bass kernel for 8 TRN2 NeuronCores (v3).

Computes  out = 0.5 * D_e ⊙ (MT.T @ (D_v ⊙ (MT @ (x @ W))))
with N=16384 nodes, E=8192 hyperedges, IN_FT=256, OUT_FT=128.

Sharding (node/data parallel): MT columns, x rows and D_e are sharded
over nodes across the 8 cores; W and D_v are replicated. The MT @ y
contraction over nodes becomes a partial sum + AllReduce; the MT.T @ z
contraction over edges is local per node shard.

v3 vs baseline:
- MT is processed in G=16 superblocks of 512 edges. For a tunable set
  of "D" superblocks the host ships a second, pre-transposed packed
  copy (mttg) so phase 1 consumes it straight from DRAM with zero PE
  transposes; the remaining "T" superblocks are PE-transposed on chip
  as before. This trades HBM bytes against PE transpose time.
- Step A computes y without any transposes (x.T shipped from host and
  used as the stationary operand).
- Phase-2 accumulates nyT in 4 persistent PSUM banks across all
  superblocks; the final 0.5*D_e scaling is a single DVE multiply that
  reads PSUM directly, and the [F, NS] result is un-transposed on the
  host. No finalize transposes.
- The AllReduce is split into 8 pipelined calls (one per 2 superblocks)
  and phase 2 lags phase 1 by LAG superblocks so each collective has
  several superblocks of PE work to hide behind.
"""

import functools
from contextlib import ExitStack

import ml_dtypes
import numpy as np

import concourse.bass as bass
import concourse.mybir as mybir
import concourse.tile as tile
from concourse import bacc
from concourse.bass_utils import run_bass_kernel_spmd
from concourse.masks import make_identity

P = 128
BF16 = mybir.dt.bfloat16
F32 = mybir.dt.float32

FULL_CFG = dict(
    N=16384, E=8192, IN=256, F=128, CORES=8, G=16, LAG=6, DMASK=0x5555)


def build_kernel(nc, cfg):
    N, E, IN, F, CORES, G = (
        cfg["N"], cfg["E"], cfg["IN"], cfg["F"], cfg["CORES"], cfg["G"])
    LAG = cfg["LAG"]
    is_d = [bool(cfg["DMASK"] >> g & 1) for g in range(G)]
    ND = sum(is_d)
    d_idx = np.cumsum([0] + is_d[:-1]).tolist()  # g -> index into mttg
    NS = N // CORES          # nodes per core
    EB = E // G              # edges per superblock
    ET = EB // P             # 128-edge chunks per superblock
    NJ = NS // P             # 128-node chunks (phase-1 contraction)
    KI = IN // P             # 128-in_ft chunks
    NQ = NS // 512           # 512-node groups (phase-2 free dim)
    NG = G // 2              # AllReduce groups (2 superblocks each)
    assert EB == 512 and NS % 512 == 0 and IN % P == 0 and F == P

    mt = nc.dram_tensor("mt", [E, NS], BF16, kind="ExternalInput").ap()
    mttg = nc.dram_tensor(
        "mttg", [max(ND, 1), P, NJ, EB], BF16, kind="ExternalInput").ap()
    xst = nc.dram_tensor("xst", [IN, NS], BF16, kind="ExternalInput").ap()
    w = nc.dram_tensor("w", [IN, F], BF16, kind="ExternalInput").ap()
    dvt = nc.dram_tensor("dvt", [P, E // P], F32, kind="ExternalInput").ap()
    detb = nc.dram_tensor("detb", [P, NS], F32, kind="ExternalInput").ap()
    out = nc.dram_tensor("out", [F, NS], F32, kind="ExternalOutput").ap()

    with tile.TileContext(nc) as tc, ExitStack() as ctx:
        consts = ctx.enter_context(tc.tile_pool(name="consts", bufs=1))
        natT_p = ctx.enter_context(tc.tile_pool(name="natT", bufs=5))
        natD_p = ctx.enter_context(tc.tile_pool(name="natD", bufs=2))
        mtTD_p = ctx.enter_context(tc.tile_pool(name="mtTD", bufs=2))
        mtTj_p = ctx.enter_context(tc.tile_pool(name="mtTj", bufs=3))
        eyp_p = ctx.enter_context(tc.tile_pool(name="eyp", bufs=2))
        eyf_p = ctx.enter_context(tc.tile_pool(name="eyf", bufs=4))
        z_p = ctx.enter_context(tc.tile_pool(name="zp", bufs=2))
        ps_tr = ctx.enter_context(tc.tile_pool(name="ps_tr", bufs=2, space="PSUM"))
        ps_ey = ctx.enter_context(tc.tile_pool(name="ps_ey", bufs=2, space="PSUM"))
        ps_ny_p = ctx.enter_context(tc.tile_pool(name="ps_ny", bufs=1, space="PSUM"))
        dram = ctx.enter_context(tc.tile_pool(name="dram", bufs=3, space="DRAM"))

        id16 = consts.tile([P, P], BF16, tag="id16")
        make_identity(nc, id16[:])

        w_sb = consts.tile([P, KI, F], BF16, tag="w")
        nc.sync.dma_start(w_sb[:], w.rearrange("(k p) f -> p k f", p=P))
        dvt_sb = consts.tile([P, E // P], F32, tag="dvt")
        nc.sync.dma_start(dvt_sb[:], dvt)
        detb_sb = consts.tile([P, NS], F32, tag="detb")
        nc.sync.dma_start(detb_sb[:], detb)
        xst_sb = consts.tile([P, KI, NS], BF16, tag="xst")
        nc.sync.dma_start(xst_sb[:], xst.rearrange("(k p) n -> p k n", p=P))

        y_sb = consts.tile([P, NJ, F], BF16, tag="y")

        # Copy-engine alternation between DVE and ACT to split PSUM->SBUF load.
        cp_state = [0]

        def copy_eng():
            cp_state[0] ^= 1
            if cp_state[0]:
                return nc.vector.tensor_copy
            return nc.scalar.copy

        # ---- Step A: y_sb[p, j, f] = y[j*128+p, f] = (x @ W) rows ---------
        # x.T is shipped from host; its 128-node column blocks are the
        # stationary operand, so no transposes are needed anywhere.
        for j in range(NJ):
            yj = ps_tr.tile([P, F], F32, tag="tr")
            for k in range(KI):
                nc.tensor.matmul(
                    yj[:],
                    lhsT=xst_sb[:, k, j * P:(j + 1) * P],
                    rhs=w_sb[:, k, :],
                    start=(k == 0),
                    stop=(k == KI - 1),
                )
            copy_eng()(y_sb[:, j, :], yj[:])

        # ---- Main pipeline over superblocks ------------------------------
        mts = {}
        eyps = {}
        eyfs = {}

        def emit_load_nat(g, pool):
            # natural layout: mt_sb[p, t, n] = MT[g*EB + t*128 + p, n]
            mt_sb = pool.tile([P, ET, NS], BF16, tag="mt")
            nc.sync.dma_start(
                mt_sb[:],
                mt[g * EB:(g + 1) * EB, :].rearrange("(t p) n -> p t n", p=P))
            mts[g] = mt_sb

        def emit_p1_D(g):
            # host-packed transposed tiles: mtT[p, j, e'] = MT[g*EB+e', j*128+p]
            mtT = mtTD_p.tile([P, NJ, EB], BF16, tag="mtTD")
            nc.sync.dma_start(mtT[:], mttg[d_idx[g]])
            eyT = ps_ey.tile([P, EB], F32, tag="ey")
            for j in range(NJ):
                nc.tensor.matmul(
                    eyT[:],
                    lhsT=y_sb[:, j, :],
                    rhs=mtT[:, j, :],
                    start=(j == 0),
                    stop=(j == NJ - 1),
                )
            _stage_eyp(g, eyT)

        def emit_p1_T(g):
            # on-chip PE transposes, software-pipelined by one j so the
            # PSUM->SBUF copy hides under the previous j's matmul.
            mt_sb = mts[g]
            eyT = ps_ey.tile([P, EB], F32, tag="ey")

            def transpose_block(j):
                tr = ps_tr.tile([P, EB], BF16, tag="tr")
                for t in range(ET):
                    nc.tensor.transpose(
                        tr[:, t * P:(t + 1) * P],
                        mt_sb[:, t, j * P:(j + 1) * P],
                        id16[:],
                    )
                mtTj = mtTj_p.tile([P, EB], BF16, tag="mtTj")
                copy_eng()(mtTj[:], tr[:])
                return mtTj

            def p1_matmul(j, mtTj):
                nc.tensor.matmul(
                    eyT[:],
                    lhsT=y_sb[:, j, :],
                    rhs=mtTj[:],
                    start=(j == 0),
                    stop=(j == NJ - 1),
                )

            prev = None
            for j in range(NJ):
                cur = transpose_block(j)
                if prev is not None:
                    p1_matmul(j - 1, prev)
                prev = cur
            p1_matmul(NJ - 1, prev)
            _stage_eyp(g, eyT)

        def _stage_eyp(g, eyT):
            k, s = g // 2, g % 2
            if s == 0:
                eyp = eyp_p.tile([P, 2, EB], BF16, tag="eyp")
                eyps[k] = eyp
            copy_eng()(eyps[k][:, s, :], eyT[:])

        def emit_ar(k):
            bin_t = dram.tile([P, 2 * EB], BF16, tag="bin")
            bout_t = dram.tile([P, 2 * EB], BF16, tag="bout")
            nc.sync.dma_start(bin_t[:], eyps[k][:])
            nc.gpsimd.collective_compute(
                "AllReduce",
                mybir.AluOpType.add,
                replica_groups=[list(range(CORES))],
                ins=[bin_t.opt()],
                outs=[bout_t.opt()],
            )
            eyf = eyf_p.tile([P, 2, ET, P], BF16, tag="eyf")
            nc.sync.dma_start(eyf[:].rearrange("p s t e -> p (s t e)"), bout_t[:])
            eyfs[k] = eyf

        def emit_p2(g):
            # z[e1, t, f] = D_v[e] * ey[e, f]; nyT[f, q*512:] accumulates
            # z.T @ MT across all superblocks in 4 persistent PSUM banks.
            k, s = g // 2, g % 2
            eyf = eyfs[k]
            z = z_p.tile([P, ET, F], BF16, tag="z")
            tr = ps_tr.tile([P, ET, P], BF16, tag="tr")
            for t in range(ET):
                nc.tensor.transpose(
                    tr[:, t, :], eyf[:, s, t, :], id16[:])
            for t in range(ET):
                nc.vector.tensor_scalar_mul(
                    z[:, t, :], tr[:, t, :],
                    dvt_sb[:, g * ET + t:g * ET + t + 1],
                )
            mt_sb = mts[g]
            for t in range(ET):
                for q in range(NQ):
                    nc.tensor.matmul(
                        ps_ny[:, q, :],
                        lhsT=z[:, t, :],
                        rhs=mt_sb[:, t, q * 512:(q + 1) * 512],
                        start=(g == 0 and t == 0),
                        stop=(g == G - 1 and t == ET - 1),
                    )
            del mts[g]

        ps_ny = ps_ny_p.tile([P, NQ, 512], F32, tag="ny")

        for g in range(G):
            if not is_d[g]:
                emit_load_nat(g, natT_p)
                emit_p1_T(g)
            else:
                emit_p1_D(g)
            # just-in-time natural load for D superblocks entering phase 2
            gl = g - LAG + 2
            if 0 <= gl < G and is_d[gl]:
                emit_load_nat(gl, natD_p)
            if g % 2 == 1:
                emit_ar(g // 2)
            if g >= LAG:
                emit_p2(g - LAG)
        for gl in range(G - LAG + 2, G):
            if is_d[gl]:
                emit_load_nat(gl, natD_p)
        for g in range(G - LAG, G):
            emit_p2(g)

        # ---- Finalize: out[f, n] = 0.5 * D_e[n] * nyT[f, n] --------------
        out_sb = consts.tile([P, NS], F32, tag="out_sb")
        nc.vector.tensor_mul(
            out_sb[:],
            ps_ny[:].rearrange("p q n -> p (q n)"),
            detb_sb[:],
        )
        nc.sync.dma_start(out, out_sb[:])

    return nc


@functools.lru_cache(maxsize=2)
def _compiled(cfg_items):
    cfg = dict(cfg_items)
    nc = bacc.Bacc(
        "TRN2",
        target_bir_lowering=False,
        debug=False,
        num_devices=cfg["CORES"],
    )
    build_kernel(nc, cfg)
    nc.compile()
    return nc


def shard_inputs(x, weight, MT, D_v_diag, D_e_diag, cfg):
    """Host-side sharding + dtype/layout prep. Returns in_maps for the 8 cores."""
    N, E, IN, F, CORES, G = (
        cfg["N"], cfg["E"], cfg["IN"], cfg["F"], cfg["CORES"], cfg["G"])
    NS = N // CORES
    EB = E // G
    NJ = NS // P
    is_d = [bool(cfg["DMASK"] >> g & 1) for g in range(G)]
    d_list = [g for g in range(G) if is_d[g]]
    bf = ml_dtypes.bfloat16
    w_b = np.ascontiguousarray(np.asarray(weight, dtype=np.float32)).astype(bf)
    x_f = np.asarray(x, dtype=np.float32)
    mt_f = np.asarray(MT, dtype=np.float32)
    dv = np.asarray(D_v_diag, dtype=np.float32)
    de = np.asarray(D_e_diag, dtype=np.float32)
    dvt = np.ascontiguousarray(dv.reshape(E // P, P).T)
    in_maps = []
    for c in range(CORES):
        sl = slice(c * NS, (c + 1) * NS)
        shard = np.ascontiguousarray(mt_f[:, sl]).astype(bf)
        # packed transposed tiles for the D superblocks:
        # mttg[d, p, j, e'] = MT[g*EB + e', j*128 + p]
        blk = shard.reshape(G, EB, NJ, P).transpose(0, 3, 2, 1)
        mttg = np.ascontiguousarray(blk[d_list]) if d_list else np.zeros(
            (1, P, NJ, EB), dtype=bf)
        detb = np.ascontiguousarray(
            np.broadcast_to(0.5 * de[sl], (P, NS)).astype(np.float32))
        in_maps.append({
            "mt": shard,
            "mttg": mttg,
            "xst": np.ascontiguousarray(x_f[sl].T).astype(bf),
            "w": w_b,
            "dvt": dvt,
            "detb": detb,
        })
    return in_maps


def _run(x, weight, MT, D_v_diag, D_e_diag, cfg=None, trace=False):
    cfg = cfg or FULL_CFG
    nc = _compiled(tuple(sorted(cfg.items())))
    in_maps = shard_inputs(x, weight, MT, D_v_diag, D_e_diag, cfg)
    res = run_bass_kernel_spmd(
        nc, in_maps, core_ids=list(range(cfg["CORES"])), trace=trace)
    out = np.concatenate(
        [np.asarray(res.results[c]["out"]).T for c in range(cfg["CORES"])],
        axis=0,
    ).astype(np.float32)
    return out, res


def kernel(x, weight, MT, D_v_diag, D_e_diag):
    out, _ = _run(x, weight, MT, D_v_diag, D_e_diag)
    return out


# revision 16
# speedup vs baseline: 1.0250x; 1.0250x over previous
"""HGNN conv distributed Bass kernel for 8 TRN2 NeuronCores (v3).

Computes  out = 0.5 * D_e ⊙ (MT.T @ (D_v ⊙ (MT @ (x @ W))))
with N=16384 nodes, E=8192 hyperedges, IN_FT=256, OUT_FT=128.

Sharding (node/data parallel): MT columns, x rows and D_e are sharded
over nodes across the 8 cores; W and D_v are replicated. The MT @ y
contraction over nodes becomes a partial sum + AllReduce; the MT.T @ z
contraction over edges is local per node shard.

v3 vs baseline:
- MT is processed in G=16 superblocks of 512 edges. For a tunable set
  of "D" superblocks the host ships a second, pre-transposed packed
  copy (mttg) so phase 1 consumes it straight from DRAM with zero PE
  transposes; the remaining "T" superblocks are PE-transposed on chip
  as before. This trades HBM bytes against PE transpose time.
- Step A computes y without any transposes (x.T shipped from host and
  used as the stationary operand).
- Phase-2 accumulates nyT in 4 persistent PSUM banks across all
  superblocks; the final 0.5*D_e scaling is a single DVE multiply that
  reads PSUM directly, and the [F, NS] result is un-transposed on the
  host. No finalize transposes.
- The AllReduce is split into 8 pipelined calls (one per 2 superblocks)
  and phase 2 lags phase 1 by LAG superblocks so each collective has
  several superblocks of PE work to hide behind.
"""

import functools
from contextlib import ExitStack

import ml_dtypes
import numpy as np

import concourse.bass as bass
import concourse.mybir as mybir
import concourse.tile as tile
from concourse import bacc
from concourse.bass_utils import run_bass_kernel_spmd
from concourse.masks import make_identity

P = 128
BF16 = mybir.dt.bfloat16
F32 = mybir.dt.float32

FULL_CFG = dict(
    N=16384, E=8192, IN=256, F=128, CORES=8, G=16, LAG=7, DMASK=0x5555,
    ARG=4)


def build_kernel(nc, cfg):
    N, E, IN, F, CORES, G = (
        cfg["N"], cfg["E"], cfg["IN"], cfg["F"], cfg["CORES"], cfg["G"])
    LAG = cfg["LAG"]
    is_d = [bool(cfg["DMASK"] >> g & 1) for g in range(G)]
    ND = sum(is_d)
    d_idx = np.cumsum([0] + is_d[:-1]).tolist()  # g -> index into mttg
    NS = N // CORES          # nodes per core
    EB = E // G              # edges per superblock
    ET = EB // P             # 128-edge chunks per superblock
    NJ = NS // P             # 128-node chunks (phase-1 contraction)
    KI = IN // P             # 128-in_ft chunks
    NQ = NS // 512           # 512-node groups (phase-2 free dim)
    ARG = cfg["ARG"]         # superblocks per AllReduce call
    NG = G // ARG            # AllReduce groups
    assert EB == 512 and NS % 512 == 0 and IN % P == 0 and F == P
    assert G % ARG == 0

    mt = nc.dram_tensor("mt", [E, NS], BF16, kind="ExternalInput").ap()
    mttg = nc.dram_tensor(
        "mttg", [max(ND, 1), P, NJ, EB], BF16, kind="ExternalInput").ap()
    xst = nc.dram_tensor("xst", [IN, NS], BF16, kind="ExternalInput").ap()
    w = nc.dram_tensor("w", [IN, F], BF16, kind="ExternalInput").ap()
    dvt = nc.dram_tensor("dvt", [P, E // P], F32, kind="ExternalInput").ap()
    detb = nc.dram_tensor("detb", [P, NS], F32, kind="ExternalInput").ap()
    out = nc.dram_tensor("out", [F, NS], F32, kind="ExternalOutput").ap()

    with tile.TileContext(nc) as tc, ExitStack() as ctx:
        consts = ctx.enter_context(tc.tile_pool(name="consts", bufs=1))
        natT_p = ctx.enter_context(tc.tile_pool(name="natT", bufs=5))
        natD_p = ctx.enter_context(tc.tile_pool(name="natD", bufs=2))
        mtTD_p = ctx.enter_context(tc.tile_pool(name="mtTD", bufs=2))
        mtTj_p = ctx.enter_context(tc.tile_pool(name="mtTj", bufs=3))
        eyp_p = ctx.enter_context(tc.tile_pool(name="eyp", bufs=2))
        eyf_p = ctx.enter_context(tc.tile_pool(name="eyf", bufs=3))
        z_p = ctx.enter_context(tc.tile_pool(name="zp", bufs=2))
        ps_tr = ctx.enter_context(tc.tile_pool(name="ps_tr", bufs=2, space="PSUM"))
        ps_ey = ctx.enter_context(tc.tile_pool(name="ps_ey", bufs=2, space="PSUM"))
        ps_ny_p = ctx.enter_context(tc.tile_pool(name="ps_ny", bufs=1, space="PSUM"))
        dram = ctx.enter_context(tc.tile_pool(name="dram", bufs=3, space="DRAM"))

        id16 = consts.tile([P, P], BF16, tag="id16")
        make_identity(nc, id16[:])

        xst_sb = consts.tile([P, KI, NS], BF16, tag="xst")
        nc.sync.dma_start(xst_sb[:], xst.rearrange("(k p) n -> p k n", p=P))
        w_sb = consts.tile([P, KI, F], BF16, tag="w")
        nc.sync.dma_start(w_sb[:], w.rearrange("(k p) f -> p k f", p=P))

        y_sb = consts.tile([P, NJ, F], BF16, tag="y")

        # Copy-engine alternation between DVE and ACT to split PSUM->SBUF load.
        cp_state = [0]

        def copy_eng():
            cp_state[0] ^= 1
            if cp_state[0]:
                return nc.vector.tensor_copy
            return nc.scalar.copy

        # ---- Step A: y_sb[p, j, f] = y[j*128+p, f] = (x @ W) rows ---------
        # x.T is shipped from host; its 128-node column blocks are the
        # stationary operand, so no transposes are needed anywhere.
        for j in range(NJ):
            yj = ps_tr.tile([P, F], F32, tag="tr")
            for k in range(KI):
                nc.tensor.matmul(
                    yj[:],
                    lhsT=xst_sb[:, k, j * P:(j + 1) * P],
                    rhs=w_sb[:, k, :],
                    start=(k == 0),
                    stop=(k == KI - 1),
                )
            copy_eng()(y_sb[:, j, :], yj[:])

        # ---- Main pipeline over superblocks ------------------------------
        mts = {}
        eyps = {}
        eyfs = {}

        def emit_load_nat(g, pool):
            # natural layout: mt_sb[p, t, n] = MT[g*EB + t*128 + p, n]
            mt_sb = pool.tile([P, ET, NS], BF16, tag="mt")
            nc.sync.dma_start(
                mt_sb[:],
                mt[g * EB:(g + 1) * EB, :].rearrange("(t p) n -> p t n", p=P))
            mts[g] = mt_sb

        def emit_p1_D(g):
            # host-packed transposed tiles: mtT[p, j, e'] = MT[g*EB+e', j*128+p]
            mtT = mtTD_p.tile([P, NJ, EB], BF16, tag="mtTD")
            nc.sync.dma_start(mtT[:], mttg[d_idx[g]])
            eyT = ps_ey.tile([P, EB], F32, tag="ey")
            for j in range(NJ):
                nc.tensor.matmul(
                    eyT[:],
                    lhsT=y_sb[:, j, :],
                    rhs=mtT[:, j, :],
                    start=(j == 0),
                    stop=(j == NJ - 1),
                )
            _stage_eyp(g, eyT)

        def emit_p1_T(g):
            # on-chip PE transposes, software-pipelined by one j so the
            # PSUM->SBUF copy hides under the previous j's matmul.
            mt_sb = mts[g]
            eyT = ps_ey.tile([P, EB], F32, tag="ey")

            def transpose_block(j):
                tr = ps_tr.tile([P, EB], BF16, tag="tr")
                for t in range(ET):
                    nc.tensor.transpose(
                        tr[:, t * P:(t + 1) * P],
                        mt_sb[:, t, j * P:(j + 1) * P],
                        id16[:],
                    )
                mtTj = mtTj_p.tile([P, EB], BF16, tag="mtTj")
                copy_eng()(mtTj[:], tr[:])
                return mtTj

            def p1_matmul(j, mtTj):
                nc.tensor.matmul(
                    eyT[:],
                    lhsT=y_sb[:, j, :],
                    rhs=mtTj[:],
                    start=(j == 0),
                    stop=(j == NJ - 1),
                )

            prev = None
            for j in range(NJ):
                cur = transpose_block(j)
                if prev is not None:
                    p1_matmul(j - 1, prev)
                prev = cur
            p1_matmul(NJ - 1, prev)
            _stage_eyp(g, eyT)

        bins = {}

        def _stage_eyp(g, eyT):
            # copy this superblock's partial into the group SBUF buffer and
            # immediately stage it into the collective's DRAM input so the
            # CC engine isn't left waiting on the whole group at AR time.
            k, s = g // ARG, g % ARG
            if s == 0:
                eyp = eyp_p.tile([P, ARG, EB], BF16, tag="eyp")
                eyps[k] = eyp
                bin_t = dram.tile([P, ARG * EB], BF16, tag="bin")
                bins[k] = bin_t
            copy_eng()(eyps[k][:, s, :], eyT[:])
            nc.sync.dma_start(
                bins[k][:, s * EB:(s + 1) * EB], eyps[k][:, s, :])

        def emit_ar(k):
            bout_t = dram.tile([P, ARG * EB], BF16, tag="bout")
            nc.gpsimd.collective_compute(
                "AllReduce",
                mybir.AluOpType.add,
                replica_groups=[list(range(CORES))],
                ins=[bins[k].opt()],
                outs=[bout_t.opt()],
            )
            eyf = eyf_p.tile([P, ARG, ET, P], BF16, tag="eyf")
            nc.sync.dma_start(eyf[:].rearrange("p s t e -> p (s t e)"), bout_t[:])
            eyfs[k] = eyf

        def emit_p2(g):
            # z[e1, t, f] = D_v[e] * ey[e, f]; nyT[f, q*512:] accumulates
            # z.T @ MT across all superblocks in 4 persistent PSUM banks.
            k, s = g // ARG, g % ARG
            eyf = eyfs[k]
            z = z_p.tile([P, ET, F], BF16, tag="z")
            tr = ps_tr.tile([P, ET, P], BF16, tag="tr")
            for t in range(ET):
                nc.tensor.transpose(
                    tr[:, t, :], eyf[:, s, t, :], id16[:])
            for t in range(ET):
                nc.vector.tensor_scalar_mul(
                    z[:, t, :], tr[:, t, :],
                    dvt_sb[:, g * ET + t:g * ET + t + 1],
                )
            mt_sb = mts[g]
            for t in range(ET):
                for q in range(NQ):
                    nc.tensor.matmul(
                        ps_ny[:, q, :],
                        lhsT=z[:, t, :],
                        rhs=mt_sb[:, t, q * 512:(q + 1) * 512],
                        start=(g == 0 and t == 0),
                        stop=(g == G - 1 and t == ET - 1),
                    )
            del mts[g]

        ps_ny = ps_ny_p.tile([P, NQ, 512], F32, tag="ny")

        dvt_sb = consts.tile([P, E // P], F32, tag="dvt")
        nc.sync.dma_start(dvt_sb[:], dvt)
        detb_sb = consts.tile([P, NS], F32, tag="detb")

        for g in range(G):
            if not is_d[g]:
                emit_load_nat(g, natT_p)
                emit_p1_T(g)
            else:
                emit_p1_D(g)
            if g == 2:
                # needed only at finalize; load mid-kernel, off the
                # startup and finalize critical paths
                nc.sync.dma_start(detb_sb[:], detb)
            # just-in-time natural load for D superblocks entering phase 2
            gl = g - LAG + 2
            if 0 <= gl < G and is_d[gl]:
                emit_load_nat(gl, natD_p)
            if g % ARG == ARG - 1:
                emit_ar(g // ARG)
            if g >= LAG:
                emit_p2(g - LAG)
        for gl in range(G - LAG + 2, G):
            if is_d[gl]:
                emit_load_nat(gl, natD_p)
        for g in range(G - LAG, G):
            emit_p2(g)

        # ---- Finalize: out[f, n] = 0.5 * D_e[n] * nyT[f, n] --------------
        out_sb = consts.tile([P, NS], F32, tag="out_sb")
        nc.vector.tensor_mul(
            out_sb[:],
            ps_ny[:].rearrange("p q n -> p (q n)"),
            detb_sb[:],
        )
        nc.sync.dma_start(out, out_sb[:])

    return nc


@functools.lru_cache(maxsize=2)
def _compiled(cfg_items):
    cfg = dict(cfg_items)
    nc = bacc.Bacc(
        "TRN2",
        target_bir_lowering=False,
        debug=False,
        num_devices=cfg["CORES"],
    )
    build_kernel(nc, cfg)
    nc.compile()
    return nc


def shard_inputs(x, weight, MT, D_v_diag, D_e_diag, cfg):
    """Host-side sharding + dtype/layout prep. Returns in_maps for the 8 cores."""
    N, E, IN, F, CORES, G = (
        cfg["N"], cfg["E"], cfg["IN"], cfg["F"], cfg["CORES"], cfg["G"])
    NS = N // CORES
    EB = E // G
    NJ = NS // P
    is_d = [bool(cfg["DMASK"] >> g & 1) for g in range(G)]
    d_list = [g for g in range(G) if is_d[g]]
    bf = ml_dtypes.bfloat16
    w_b = np.ascontiguousarray(np.asarray(weight, dtype=np.float32)).astype(bf)
    x_f = np.asarray(x, dtype=np.float32)
    mt_f = np.asarray(MT, dtype=np.float32)
    dv = np.asarray(D_v_diag, dtype=np.float32)
    de = np.asarray(D_e_diag, dtype=np.float32)
    dvt = np.ascontiguousarray(dv.reshape(E // P, P).T)
    in_maps = []
    for c in range(CORES):
        sl = slice(c * NS, (c + 1) * NS)
        shard = np.ascontiguousarray(mt_f[:, sl]).astype(bf)
        # packed transposed tiles for the D superblocks:
        # mttg[d, p, j, e'] = MT[g*EB + e', j*128 + p]
        blk = shard.reshape(G, EB, NJ, P).transpose(0, 3, 2, 1)
        mttg = np.ascontiguousarray(blk[d_list]) if d_list else np.zeros(
            (1, P, NJ, EB), dtype=bf)
        detb = np.ascontiguousarray(
            np.broadcast_to(0.5 * de[sl], (P, NS)).astype(np.float32))
        in_maps.append({
            "mt": shard,
            "mttg": mttg,
            "xst": np.ascontiguousarray(x_f[sl].T).astype(bf),
            "w": w_b,
            "dvt": dvt,
            "detb": detb,
        })
    return in_maps


def _run(x, weight, MT, D_v_diag, D_e_diag, cfg=None, trace=False):
    cfg = cfg or FULL_CFG
    nc = _compiled(tuple(sorted(cfg.items())))
    in_maps = shard_inputs(x, weight, MT, D_v_diag, D_e_diag, cfg)
    res = run_bass_kernel_spmd(
        nc, in_maps, core_ids=list(range(cfg["CORES"])), trace=trace)
    out = np.concatenate(
        [np.asarray(res.results[c]["out"]).T for c in range(cfg["CORES"])],
        axis=0,
    ).astype(np.float32)
    return out, res


def kernel(x, weight, MT, D_v_diag, D_e_diag):
    out, _ = _run(x, weight, MT, D_v_diag, D_e_diag)
    return out
